# revision 17
# baseline (speedup 1.0000x reference)
"""Trainium2 Bass kernel for ConditionalLinearAttention.

Math (per batch element b, shapes hardcoded):
  xf  = x[b].reshape(256, 4096)
  cf  = cond_emb[b].reshape(512, 128)
  kv  = Wcond @ cf                      # (1024, 128)
  k   = softmax(kv[:512], per-row over the 128 cond positions)
  v   = kv[512:]
  ctx[h] = k_h @ v_h.T                  # (64, 64) per head h
  out = Wout @ apply(ctx) @ Wq @ xf + b_out

Key optimization: ctx is tiny and per-batch, so the whole attention folds
into one per-batch matrix W_comb = Wout @ ctxE @ Wq (256x256); the spatial
dimension then sees ONE (256x256)@(256x4096) GEMM instead of three big
GEMMs. Sharding: data-parallel over batch, one batch element per core.

Device dataflow per core (P=128 partitions). Softmax normalization is
folded into the context rows (ctx_norm[d,e] = (1/Z[hd]) sum_m exp(kv_k[hd,m])
v[he,m]) so exp stays un-normalized and no on-chip transpose is needed:

  kvT (m,o)   = sum_j cf[:,j,:].T @ WcondT[:,j,:]      (8 MM, N=512)
  expkT       = exp(kvT[:, :512])                      (1 ACT op)
  Z cols      = expkT[:,128i:].T @ ones                (4 MM, N=1)
  ctx pair i  = expkT[:,128i:].T @ vT[:,128i:]         (4 MM f32, N=128)
                -> diagonal 64x64 blocks scaled by 1/Z into blockdiag ctx_bd
  A[:,i,:]    = ctx_bd[:,i,:].T @ Wq[:,i,:]            (4 MM, N=256)
  W_combT     = sum_kk A[:,kk,mc].T @ WoutT[:,kk,:]    (8 MM, N=256)
  OUT         = sum_ck W_combT[:,ck,mo].T @ x[:,ck,nt] + b  (32 MM, N=512)

Matmul dtypes are configurable per stream group (env KERNEL_*DT):
float32r (same bytes as fp32, single-pass PE matmul at N>=256) or bfloat16
(halves the DMA traffic, the kernel's binding resource).
"""

import os

import numpy as np

B = 8
C = 256
N_SPATIAL = 4096  # 64*64
P = 128
N_CORES = 8

# dtype knobs: "f32r" or "bf16"
XDT = os.environ.get("KERNEL_XDT", "f32r")    # x + W_comb (phase-2 GEMM)
CDT = os.environ.get("KERNEL_CDT", "f32r")    # cond_emb + WcondT (kv projection)
WDT = os.environ.get("KERNEL_WDT", "f32r")    # Wq/WoutT + ctx/A (W_comb build)
ODT = os.environ.get("KERNEL_ODT", "f32")     # output stream
WARM = int(os.environ.get("KERNEL_WARM", "8"))  # PE warmup matmuls

_CACHE = {}
LAST_RESULTS = None  # BassKernelResults of the most recent run (for test.py)


def _mdt(name):
    import concourse.mybir as mybir

    return {"f32r": mybir.dt.float32r, "bf16": mybir.dt.bfloat16,
            "f32": mybir.dt.float32}[name]


def _npdt(name):
    import ml_dtypes

    return {"f32r": np.float32, "bf16": ml_dtypes.bfloat16,
            "f32": np.float32}[name]


def _build_nc():
    import concourse.bacc as bacc
    import concourse.mybir as mybir
    import concourse.tile as tile

    fp32 = mybir.dt.float32
    xdt, cdt, wdt, odt = _mdt(XDT), _mdt(CDT), _mdt(WDT), _mdt(ODT)
    AF = mybir.ActivationFunctionType

    nc = bacc.Bacc("TRN2", target_bir_lowering=False, debug=False,
                   num_devices=N_CORES)

    x_t = nc.dram_tensor("x", [C, N_SPATIAL], xdt, kind="ExternalInput").ap()
    cf_t = nc.dram_tensor("cf", [512, 128], cdt, kind="ExternalInput").ap()
    wct_t = nc.dram_tensor("wcondT", [512, 1024], cdt, kind="ExternalInput").ap()
    wq_t = nc.dram_tensor("wq", [512, 256], wdt, kind="ExternalInput").ap()
    wot_t = nc.dram_tensor("woutT", [512, 256], wdt, kind="ExternalInput").ap()
    bias_t = nc.dram_tensor("bias", [256, 1], fp32, kind="ExternalInput").ap()
    out_t = nc.dram_tensor("out", [C, N_SPATIAL], odt, kind="ExternalOutput").ap()

    NC_ = 4          # x chunks of 1024 spatial positions
    CW = N_SPATIAL // NC_
    NW = 512         # matmul moving width

    def zero_fill(ap, dtype):
        if dtype == mybir.dt.float32r:
            nc.vector.memset(ap.bitcast(mybir.dt.uint32), 0)
        else:
            nc.vector.memset(ap, 0.0)

    with tile.TileContext(nc) as tc:
        with (
            tc.tile_pool(name="main", bufs=1) as mainp,
            tc.tile_pool(name="work", bufs=2) as workp,
            tc.tile_pool(name="outp", bufs=6) as outp,
            tc.tile_pool(name="ps", bufs=3, space="PSUM") as psp,
            tc.tile_pool(name="psO", bufs=5, space="PSUM") as psO,
        ):
            xr = x_t.rearrange("(ck p) n -> p ck n", p=P)        # (128, 2, 4096)
            cfr = cf_t.rearrange("(ko p) m -> p ko m", p=P)      # (128, 4, 128)
            wcr = wct_t.rearrange("(ko p) o -> p ko o", p=P)     # (128, 4, 1024)
            wqr = wq_t.rearrange("(i p) c -> p i c", p=P)        # (128, 4, 256)
            wor = wot_t.rearrange("(kk p) o -> p kk o", p=P)     # (128, 4, 256)
            br = bias_t.rearrange("(mo p) one -> p mo one", p=P) # (128, 2, 1)
            outr = out_t.rearrange("(mo p) n -> p mo n", p=P)    # (128, 2, 4096)

            # warmup operand tiles first: junk matmuls must be runnable the
            # moment the engines clear the entry rendezvous
            wl = mainp.tile([P, P], mybir.dt.bfloat16)
            nc.gpsimd.memset(wl, 0.0)
            wz = mainp.tile([P, 512], mybir.dt.bfloat16)
            nc.vector.memset(wz, 0.0)

            # --- input DMAs, critical-path order on the sync (SP) HWDGE ring
            cf_sb = mainp.tile([P, 4, 128], cdt)
            nc.sync.dma_start(cf_sb, cfr)
            wckA = mainp.tile([P, 2, 512], cdt)
            nc.sync.dma_start(wckA, wcr[:, 0:2, 0:512])
            wckB = mainp.tile([P, 2, 512], cdt)
            nc.sync.dma_start(wckB, wcr[:, 2:4, 0:512])
            wcvA = mainp.tile([P, 2, 512], cdt)
            nc.sync.dma_start(wcvA, wcr[:, 0:2, 512:1024])
            wcvB = mainp.tile([P, 2, 512], cdt)
            nc.sync.dma_start(wcvB, wcr[:, 2:4, 512:1024])
            wq_sb = mainp.tile([P, 4, 256], wdt)
            nc.sync.dma_start(wq_sb, wqr)
            wo_sb = mainp.tile([P, 4, 256], wdt)
            nc.sync.dma_start(wo_sb, wor)
            x_sb = []
            for cc in range(NC_):
                t = mainp.tile([P, 2, CW], xdt, tag=f"x{cc}")
                nc.sync.dma_start(t, xr[:, :, CW * cc:CW * (cc + 1)])
                x_sb.append(t)
            # bias: 256 tiny strided descriptors -> keep off the SP ring
            bias_sb = mainp.tile([P, 2, 1], fp32)
            nc.gpsimd.dma_start(bias_sb, br)
            ones_sb = mainp.tile([P, 1], fp32)
            nc.vector.memset(ones_sb, 1.0)

            # PE warmup: junk matmuls with no DMA deps fill the otherwise-idle
            # input-DMA window so HAM unthrottles (1.2 -> 2.4 GHz) before the
            # real matmuls start
            def keep_warm(n):
                for _ in range(n):
                    pj = psO.tile([P, 512], fp32, tag="O")
                    nc.tensor.matmul(pj, wl, wz, start=True, stop=True)

            keep_warm(WARM)

            # --- phase 1: per-batch W_comb (256x256) ---
            # kvT (cond position m on partitions): k half and v half
            pkv = psp.tile([P, 512], fp32, tag="p1")
            for j in range(4):
                wck_j = wckA[:, j, :] if j < 2 else wckB[:, j - 2, :]
                nc.tensor.matmul(pkv, cf_sb[:, j, :], wck_j,
                                 start=(j == 0), stop=(j == 3))
            expkT = mainp.tile([P, 512], fp32)
            nc.scalar.activation(out=expkT, in_=pkv, func=AF.Exp)

            pvv = psp.tile([P, 512], fp32, tag="p1")
            for j in range(4):
                wcv_j = wcvA[:, j, :] if j < 2 else wcvB[:, j - 2, :]
                nc.tensor.matmul(pvv, cf_sb[:, j, :], wcv_j,
                                 start=(j == 0), stop=(j == 3))
            vT = mainp.tile([P, 512], fp32)
            nc.vector.tensor_copy(out=vT, in_=pvv)

            # softmax denominators as columns: Z[hd] = sum_m expkT[m, hd]
            rcol = []
            for i in range(4):
                pz = psp.tile([P, 1], fp32, tag="p1")
                nc.tensor.matmul(pz, expkT[:, 128 * i:128 * (i + 1)], ones_sb,
                                 start=True, stop=True)
                rc = workp.tile([P, 1], fp32, tag=f"r{i}")
                nc.vector.reciprocal(rc, pz)
                rcol.append(rc)
            keep_warm(2)

            # per-head-pair context; scale rows by 1/Z while extracting the
            # diagonal 64x64 blocks into the block-diagonal layout
            ctx_bd = mainp.tile([P, 4, 128], wdt)
            zero_fill(ctx_bd, wdt)
            for i in range(4):
                pc = psp.tile([P, 128], fp32, tag="p1")
                nc.tensor.matmul(pc, expkT[:, 128 * i:128 * (i + 1)],
                                 vT[:, 128 * i:128 * (i + 1)], start=True, stop=True)
                nc.vector.tensor_scalar_mul(ctx_bd[0:64, i, 0:64],
                                            pc[0:64, 0:64], rcol[i][0:64])
                nc.vector.tensor_scalar_mul(ctx_bd[64:128, i, 64:128],
                                            pc[64:128, 64:128], rcol[i][64:128])
            keep_warm(2)

            # A[he, c] = blockdiag(ctx).T @ Wq  (k-tile i = head pair i)
            A_sb = mainp.tile([P, 4, 256], wdt)
            for i in range(4):
                pa = psp.tile([P, 256], fp32, tag="p1")
                nc.tensor.matmul(pa, ctx_bd[:, i, :], wq_sb[:, i, :],
                                 start=True, stop=True)
                nc.vector.tensor_copy(out=A_sb[:, i, :], in_=pa)
            keep_warm(2)

            # W_combT[c, o] = sum_he A[he, c] * WoutT[he, o]
            wc_sb = mainp.tile([P, 2, 256], xdt)
            for mc in range(2):
                pw = psp.tile([P, 256], fp32, tag="p1")
                for kk in range(4):
                    nc.tensor.matmul(pw, A_sb[:, kk, 128 * mc:128 * (mc + 1)],
                                     wo_sb[:, kk, :], start=(kk == 0), stop=(kk == 3))
                nc.vector.tensor_copy(out=wc_sb[:, mc, :], in_=pw)
            keep_warm(2)

            # --- phase 2: OUT = W_comb @ xf + bias, streamed over x chunks
            for cc in range(NC_):
                for sub in range(CW // NW):
                    nt = cc * (CW // NW) + sub
                    ot = outp.tile([P, 2, NW], odt, tag="osb")
                    for mo in range(2):
                        po = psO.tile([P, NW], fp32, tag="O")
                        for ck in range(2):
                            nc.tensor.matmul(
                                po, wc_sb[:, ck, 128 * mo:128 * (mo + 1)],
                                x_sb[cc][:, ck, NW * sub:NW * (sub + 1)],
                                start=(ck == 0), stop=(ck == 1))
                        if mo == 0:
                            nc.scalar.activation(out=ot[:, mo, :], in_=po,
                                                 func=AF.Identity,
                                                 bias=bias_sb[:, mo, :], scale=1.0)
                        else:
                            nc.vector.tensor_scalar_add(out=ot[:, mo, :], in0=po,
                                                        scalar1=bias_sb[:, mo, :])
                    eng = nc.scalar if nt % 2 == 0 else nc.sync
                    eng.dma_start(outr[:, :, NW * nt:NW * (nt + 1)], ot)

    nc.compile()
    return nc


def kernel(x, cond_emb, Wq, Wcond, Wout, b_out):
    from concourse.bass_utils import run_bass_kernel_spmd

    global LAST_RESULTS

    if "nc" not in _CACHE:
        _CACHE["nc"] = _build_nc()
    nc = _CACHE["nc"]

    xnp, cnp, wnp = _npdt(XDT), _npdt(CDT), _npdt(WDT)
    xf = np.ascontiguousarray(x.reshape(B, C, N_SPATIAL)).astype(xnp)
    cf = np.ascontiguousarray(cond_emb.reshape(B, 512, 128)).astype(cnp)
    wcondT = np.ascontiguousarray(Wcond.T).astype(cnp)
    wq = np.ascontiguousarray(Wq).astype(wnp)
    woutT = np.ascontiguousarray(Wout.T).astype(wnp)
    bias = np.ascontiguousarray(b_out.reshape(256, 1)).astype(np.float32)

    in_maps = [
        {
            "x": np.ascontiguousarray(xf[b]),
            "cf": np.ascontiguousarray(cf[b]),
            "wcondT": wcondT,
            "wq": wq,
            "woutT": woutT,
            "bias": bias,
        }
        for b in range(B)
    ]

    trace = bool(int(os.environ.get("KERNEL_TRACE", "0")))
    res = run_bass_kernel_spmd(nc, in_maps, core_ids=list(range(N_CORES)),
                               trace=trace)
    LAST_RESULTS = res
    out = np.stack([res.results[b]["out"] for b in range(B)])
    return out.reshape(B, C, 64, 64).astype(np.float32)


if __name__ == "__main__":
    xs = np.random.RandomState(0)
    ins = {
        "x": xs.randn(8, 256, 64, 64).astype(np.float32),
        "cond_emb": xs.randn(8, 512, 1, 128).astype(np.float32),
        "Wq": (xs.randn(512, 256) * 0.05).astype(np.float32),
        "Wcond": (xs.randn(1024, 512) * 0.05).astype(np.float32),
        "Wout": (xs.randn(256, 512) * 0.05).astype(np.float32),
        "b_out": np.zeros(256, np.float32),
    }
    o = kernel(**ins)
    print("ran, shape", o.shape)
